# revision 1
# baseline (speedup 1.0000x reference)
"""BrainRNN forward pass on 8 TRN2 NeuronCores (Bass/Tile, SPMD).

Strategy (tensor-parallel over output neurons, fp32 exact):
  - Each block's 1024 output neurons are row-sharded 128/core; masks are
    folded into weights on the host (memory-bound: halves HBM traffic).
  - Matmuls run with the (streamed) weights as the MOVING operand and the
    activations as the 64-wide stationary operand: psum[b, m] += sum_k
    actT[k, b] * W.T[k, m].  fp32 moving runs ~2 cyc/row vs ~4x slower
    weights-stationary LDWEIGHTS.
  - Adjacent blocks share one [64, 256] PSUM accumulator (bank budget);
    per-block biases enter via a K=1 "ones x biasrow" matmul that opens
    each accumulator.
  - Chain per block: sigmoid (PSUM->SBUF), PE transpose to [128(m), 64(b)],
    DVE copy, gpsimd bounce DMA, AllGather over 8 cores, 3-way staggered
    unpack on the scalar ring.
  - DMA emission order tracks the chain's deadlines (rec block j due at
    sigmoid_j); matmul emission back-fills AllGather wait windows with
    ready work (later-pair skip contributions) to keep the PE warm.
  - Output block: each core contracts its own 128 rows of cur7 against
    W_out[:, rows].T (N=512), then one ReduceScatter yields each core an
    8-row batch shard of the [64, 512] output.
"""

import numpy as np

N = 8192
W = 1024
L = 8
B = 64
IN = 512
OUT = 512
NCORES = 8
RP = W // NCORES        # 128 rows per core per block

_BUILT = None


def _pack(A):
    """[M, K] -> [128, (K/128)*M] with packed[p, k*M+m] = A[m, k*128+p].

    Chunk kidx is A[:, kidx*128:(kidx+1)*128].T, i.e. [K=128(part), M(free)].
    """
    M, K = A.shape
    nk = K // 128
    return np.ascontiguousarray(
        A.reshape(M, nk, 128).transpose(2, 1, 0).reshape(128, nk * M)
    )


def _interleave(Pa, Pb):
    """Two packed [128, nk*128] -> [128, nk*256] with per-chunk interleave."""
    nk = Pa.shape[1] // 128
    out = np.empty((128, nk, 2, 128), np.float32)
    out[:, :, 0, :] = Pa.reshape(128, nk, 128)
    out[:, :, 1, :] = Pb.reshape(128, nk, 128)
    return np.ascontiguousarray(out.reshape(128, nk * 256))


def _build():
    import concourse.bass as bass
    import concourse.bacc as bacc
    import concourse.mybir as mybir
    import concourse.tile as tile

    fp32 = mybir.dt.float32
    AF = mybir.ActivationFunctionType

    nc = bacc.Bacc(
        "TRN2",
        target_bir_lowering=False,
        debug=False,
        enable_asserts=False,
        num_devices=NCORES,
    )

    t_hT = nc.dram_tensor("hT", [128, 64 * B], fp32, kind="ExternalInput")
    t_xT = nc.dram_tensor("xT", [128, 4 * B], fp32, kind="ExternalInput")
    t_win = nc.dram_tensor("win", [128, 4 * RP], fp32, kind="ExternalInput")
    t_rec = {
        j: nc.dram_tensor(f"rec{j}", [128, 64 * 128], fp32, kind="ExternalInput")
        for j in range(7)
    }
    t_hid = nc.dram_tensor("hid", [7, 128, 8 * RP], fp32, kind="ExternalInput")
    # skip{q}_{c}: pair q=(2q, 2q+1), cur-source block c.  c < 2q: both
    # blocks interleaved [128, 8*256]; c == 2q: later block only [128, 8*128].
    t_skip = {}
    for q in range(1, 4):
        a = 2 * q
        for c in range(a + 1):
            wdt = 8 * 256 if c < a else 8 * 128
            t_skip[(q, c)] = nc.dram_tensor(
                f"skip{q}_{c}", [128, wdt], fp32, kind="ExternalInput"
            )
    t_biasrow = nc.dram_tensor("biasrow", [1, 4 * 256], fp32, kind="ExternalInput")
    t_wout = nc.dram_tensor("wout", [128, 512], fp32, kind="ExternalInput")
    t_boutrow = nc.dram_tensor("boutrow", [1, 512], fp32, kind="ExternalInput")
    t_ones = nc.dram_tensor("ones", [1, B], fp32, kind="ExternalInput")
    t_ident = nc.dram_tensor("ident", [B, B], fp32, kind="ExternalInput")
    t_out = nc.dram_tensor("out", [8, 512], fp32, kind="ExternalOutput")

    rg = [list(range(NCORES))]
    qof = lambda j: j // 2          # pair index of block j
    side = lambda j: j % 2          # column side within pair tile

    with tile.TileContext(nc) as tc:
        with (
            tc.tile_pool(name="const", bufs=1) as constp,
            tc.tile_pool(name="wrec", bufs=3) as wrecp,
            tc.tile_pool(name="whid", bufs=1) as whidp,
            tc.tile_pool(name="wskip", bufs=1) as wskipp,
            tc.tile_pool(name="curs", bufs=1) as curp,
            tc.tile_pool(name="psum", bufs=1, space="PSUM") as psump,
            tc.tile_pool(name="dram", bufs=3, space="DRAM") as dramp,
        ):
            # ---- persistent inputs -------------------------------------
            hT_sb = constp.tile([128, 64 * B], fp32, name="hT_sb", tag="hT")
            nc.sync.dma_start(out=hT_sb, in_=t_hT[:, :])
            xT_sb = constp.tile([128, 4 * B], fp32, name="xT_sb", tag="xT")
            nc.sync.dma_start(out=xT_sb, in_=t_xT[:, :])
            win_sb = constp.tile([128, 4 * RP], fp32, name="win_sb", tag="win")
            nc.sync.dma_start(out=win_sb, in_=t_win[:, :])
            biasrow_sb = constp.tile([1, 4 * 256], fp32, name="biasrow_sb", tag="br")
            nc.sync.dma_start(out=biasrow_sb, in_=t_biasrow[:, :])
            wout_sb = constp.tile([128, 512], fp32, name="wout_sb", tag="wout")
            nc.sync.dma_start(out=wout_sb, in_=t_wout[:, :])
            boutrow_sb = constp.tile([1, 512], fp32, name="boutrow_sb", tag="bo")
            nc.sync.dma_start(out=boutrow_sb, in_=t_boutrow[:, :])
            ones_sb = constp.tile([1, B], fp32, name="ones_sb", tag="ones")
            nc.sync.dma_start(out=ones_sb, in_=t_ones[:, :])
            ident_sb = constp.tile([B, B], fp32, name="ident_sb", tag="ident")
            nc.sync.dma_start(out=ident_sb, in_=t_ident[:, :])

            psA = [
                psump.tile([64, 256], fp32, name=f"psA{q}", tag=f"psA{q}")
                for q in range(4)
            ]
            curT = [None] * 8

            # bias-init: psA[q] = ones.T @ biasrow[q]  (start=True opens group)
            for q in range(4):
                nc.tensor.matmul(
                    psA[q],
                    lhsT=ones_sb[:, :],
                    rhs=biasrow_sb[:, q * 256:(q + 1) * 256],
                    start=True,
                    stop=False,
                )

            def rec_blk(j):
                """Stream block j's rec weights (1MB tiles, 512KB DMAs)."""
                q, s = qof(j), side(j)
                for h in range(4):
                    rt = wrecp.tile([128, 2048], fp32, name=f"rec{j}h{h}", tag="rec")
                    for i in range(2):
                        nc.sync.dma_start(
                            out=rt[:, i * 1024:(i + 1) * 1024],
                            in_=t_rec[j][:, h * 2048 + i * 1024:
                                         h * 2048 + (i + 1) * 1024],
                        )
                    for k in range(16):
                        kg = h * 16 + k
                        nc.tensor.matmul(
                            psA[q][:, s * 128:(s + 1) * 128],
                            lhsT=hT_sb[:, kg * B:(kg + 1) * B],
                            rhs=rt[:, k * 128:(k + 1) * 128],
                            start=False,
                            stop=False,
                        )

            def chain_tail(j):
                """sigmoid -> transpose -> AllGather -> staggered unpack."""
                q, s = qof(j), side(j)
                cp = curp.tile([64, 128], fp32, name=f"cpart{j}", tag="cpart", bufs=2)
                nc.scalar.activation(cp, psA[q][:, s * 128:(s + 1) * 128], AF.Sigmoid)
                pt = psump.tile([128, B], fp32, name=f"pt{j}", tag="pt", bufs=2)
                nc.tensor.transpose(pt, cp, ident_sb[:, :])
                ptsb = curp.tile([128, B], fp32, name=f"ptsb{j}", tag="ptsb", bufs=2)
                nc.vector.tensor_copy(ptsb, pt)
                agin = dramp.tile([128, B], fp32, name=f"agin{j}", tag="agin")
                agout = dramp.tile([W, B], fp32, name=f"agout{j}", tag="agout")
                nc.gpsimd.dma_start(out=agin, in_=ptsb)
                nc.gpsimd.collective_compute(
                    "AllGather",
                    mybir.AluOpType.bypass,
                    replica_groups=rg,
                    ins=[agin.opt()],
                    outs=[agout.opt()],
                )
                dst = curp.tile([128, 8 * B], fp32, name=f"curT{j}", tag="curT",
                                bufs=6)
                # gpsimd just observed the collective's completion sem: it can
                # issue chunks 0-3 with no cross-engine hop; scalar does 4-7
                # in parallel on its own ring.
                for eng, kk in ((nc.gpsimd, 0), (nc.scalar, 4)):
                    eng.dma_start(
                        out=dst[:, kk * B:(kk + 4) * B].rearrange(
                            "p (k b) -> p k b", k=4
                        ),
                        in_=agout[kk * 128:(kk + 4) * 128, :].rearrange(
                            "(k p) b -> p k b", p=128
                        ),
                    )
                curT[j] = dst

            hid_tiles = {}

            def hid_tile(j):
                # resident (distinct tag): allocation can never stall the ring
                ht = whidp.tile([128, 8 * RP], fp32, name=f"hid{j}", tag=f"hid{j}")
                nc.sync.dma_start(out=ht, in_=t_hid[j - 1, :, :])
                hid_tiles[j] = ht

            def hid_mms(j):
                """cur_{j-1} @ W_hid[j-1].T into block j's psum columns."""
                q, s = qof(j), side(j)
                if j not in hid_tiles:
                    hid_tile(j)
                ht = hid_tiles[j]
                for kk in range(8):
                    nc.tensor.matmul(
                        psA[q][:, s * 128:(s + 1) * 128],
                        lhsT=curT[j - 1][:, kk * B:(kk + 1) * B],
                        rhs=ht[:, kk * RP:(kk + 1) * RP],
                        start=False,
                        stop=(s == 1 and kk == 7),
                    )

            skip_tiles = {}

            def skip_tile(q, c):
                """Allocate + DMA pair q's skip weights sourced from cur_c."""
                a = 2 * q
                wide = 256 if c < a else 128
                st = wskipp.tile(
                    [128, 8 * wide], fp32, name=f"skip{q}_{c}t", tag=f"skip{q}_{c}"
                )
                half = 4 * wide
                for i in range(2):
                    nc.sync.dma_start(
                        out=st[:, i * half:(i + 1) * half],
                        in_=t_skip[(q, c)][:, i * half:(i + 1) * half],
                    )
                skip_tiles[(q, c)] = st

            def skip_one(q, c, part=None):
                """MMs for pair q's skip from cur_c.  part: None=all columns,
                'lo'=first block's 128 cols, 'hi'=second block's 128 cols."""
                a = 2 * q
                wide = 256 if c < a else 128
                if (q, c) not in skip_tiles:
                    skip_tile(q, c)
                st = skip_tiles[(q, c)]
                off = 0 if c < a else 128
                rlo, rwide = 0, wide
                if part == "lo" and wide == 256:
                    rlo, rwide = 0, 128
                elif part == "hi" and wide == 256:
                    rlo, rwide = 128, 128
                for kk in range(8):
                    nc.tensor.matmul(
                        psA[q][:, off + rlo:off + rlo + rwide],
                        lhsT=curT[c][:, kk * B:(kk + 1) * B],
                        rhs=st[:, kk * wide + rlo:kk * wide + rlo + rwide],
                        start=False,
                        stop=False,
                    )

            # ---- block 0: x @ W_in.T + h @ Wrec(block0).T (+bias) ------
            for kk in range(4):
                nc.tensor.matmul(
                    psA[0][:, 0:128],
                    lhsT=xT_sb[:, kk * B:(kk + 1) * B],
                    rhs=win_sb[:, kk * RP:(kk + 1) * RP],
                    start=False,
                    stop=False,
                )
            rec_blk(0)
            chain_tail(0)

            hid_tile(1)
            rec_blk(1)                  # AG_0 window work
            hid_mms(1)
            chain_tail(1)

            skip_tile(1, 1)
            skip_tile(1, 0)
            hid_tile(2)
            rec_blk(2)                  # AG_1 window work
            skip_one(1, 0)
            skip_one(1, 1, part="lo")   # urgent: block 2's last skip source
            hid_mms(2)
            chain_tail(2)

            skip_tile(1, 2)
            hid_tile(3)
            skip_tile(2, 0)
            rec_blk(3)                  # AG_2 window work
            skip_one(1, 1, part="hi")
            skip_one(1, 2)              # urgent for block 3
            hid_mms(3)
            chain_tail(3)

            rec_blk(4)                  # AG_3 window work
            skip_tile(2, 1)
            hid_tile(4)
            skip_tile(2, 2)
            skip_tile(2, 3)
            skip_one(2, 0)
            skip_one(2, 1)
            skip_one(2, 2)
            skip_one(2, 3, part="lo")   # urgent for block 4
            hid_mms(4)
            chain_tail(4)

            rec_blk(5)                  # AG_4 window work
            hid_tile(5)
            skip_tile(3, 0)
            skip_tile(2, 4)
            skip_one(3, 0)
            skip_one(2, 3, part="hi")
            skip_one(2, 4)              # urgent for block 5
            hid_mms(5)
            chain_tail(5)

            rec_blk(6)                  # AG_5 window work
            skip_tile(3, 1)
            skip_tile(3, 2)
            skip_tile(3, 3)
            skip_tile(3, 4)
            skip_tile(3, 5)
            skip_one(3, 1)
            skip_one(3, 2)
            skip_one(3, 3)
            skip_one(3, 4)
            skip_one(3, 5, part="lo")   # urgent for block 6
            hid_mms(6)
            chain_tail(6)

            skip_tile(3, 6)
            hid_tile(7)
            skip_one(3, 5, part="hi")   # AG_6 window work
            skip_one(3, 6)              # urgent for block 7
            hid_mms(7)

            # block 7 tail: sigmoid -> transpose -> local out matmul -> RS
            cp7 = curp.tile([64, 128], fp32, name="cpart7", tag="cpart", bufs=2)
            nc.scalar.activation(cp7, psA[3][:, 128:256], AF.Sigmoid)
            pt7 = psump.tile([128, B], fp32, name="pt7", tag="pt", bufs=2)
            nc.tensor.transpose(pt7, cp7, ident_sb[:, :])
            cur7T_sb = curp.tile([128, B], fp32, name="cur7T_sb", tag="c7T")
            nc.vector.tensor_copy(cur7T_sb, pt7)

            pso = psump.tile([64, 512], fp32, name="pso", tag="pso")
            nc.tensor.matmul(
                pso, lhsT=ones_sb[:, :], rhs=boutrow_sb[:, :], start=True, stop=False
            )
            nc.tensor.matmul(
                pso, lhsT=cur7T_sb, rhs=wout_sb[:, :], start=False, stop=True
            )
            out_sb = curp.tile([64, 512], fp32, name="out_sb", tag="out_sb")
            nc.vector.tensor_copy(out_sb, pso)
            rs_in = dramp.tile([64, 512], fp32, name="rs_in", tag="rs_in")
            rs_out = dramp.tile([8, 512], fp32, name="rs_out", tag="rs_out")
            nc.scalar.dma_start(out=rs_in, in_=out_sb)
            nc.gpsimd.collective_compute(
                "ReduceScatter",
                mybir.AluOpType.add,
                replica_groups=rg,
                ins=[rs_in.opt()],
                outs=[rs_out.opt()],
            )
            nc.scalar.dma_start(out=t_out[:, :], in_=rs_out)

    nc.compile()
    return nc


def _get_nc():
    global _BUILT
    if _BUILT is None:
        _BUILT = _build()
    return _BUILT


def make_in_maps(x, hidden_states, W_in, b_in, W_hid, b_hid, W_rec, W_skip,
                 W_out, b_out, mask_hid, mask_rec, mask_skip):
    x = np.asarray(x, np.float32)
    h = np.asarray(hidden_states, np.float32)
    W_in = np.asarray(W_in, np.float32)
    b_in = np.asarray(b_in, np.float32)
    W_out = np.asarray(W_out, np.float32)
    b_out = np.asarray(b_out, np.float32)
    Wh = np.asarray(W_hid, np.float32) * np.asarray(mask_hid, np.float32)
    Wr = np.asarray(W_rec, np.float32) * np.asarray(mask_rec, np.float32)
    Ws = np.asarray(W_skip, np.float32) * np.asarray(mask_skip, np.float32)
    b_hid = np.asarray(b_hid, np.float32)

    hT = _pack(h)
    xT = _pack(x)
    ones = np.ones((1, B), np.float32)
    ident = np.eye(B, dtype=np.float32)
    # every core's partial includes the bias and ReduceScatter sums them
    boutrow = np.ascontiguousarray(b_out[None, :]) / NCORES

    in_maps = []
    for c_ in range(NCORES):
        R = slice(c_ * RP, (c_ + 1) * RP)
        biases = [b_in[R]] + [b_hid[i, R] for i in range(7)]
        biasrow = np.zeros((1, 4 * 256), np.float32)
        for j in range(8):
            biasrow[0, j * 128:(j + 1) * 128] = biases[j]
        m = {
            "hT": hT,
            "xT": xT,
            "win": _pack(W_in[R]),
            "hid": np.stack([_pack(Wh[i, R]) for i in range(7)]),
            "biasrow": biasrow,
            "wout": np.ascontiguousarray(W_out[:, R].T),
            "boutrow": boutrow,
            "ones": ones,
            "ident": ident,
        }
        for j in range(7):
            m[f"rec{j}"] = _pack(Wr[j, R])
        packs = {j: _pack(Ws[j - 2, R, :j * W]).reshape(128, j * 8, 128)
                 for j in range(2, 8)}
        for q in range(1, 4):
            a = 2 * q
            for c in range(a + 1):
                if c < a:
                    Pa = packs[a][:, c * 8:(c + 1) * 8, :].reshape(128, 8 * 128)
                    Pb = packs[a + 1][:, c * 8:(c + 1) * 8, :].reshape(128, 8 * 128)
                    m[f"skip{q}_{c}"] = _interleave(
                        np.ascontiguousarray(Pa), np.ascontiguousarray(Pb)
                    )
                else:
                    m[f"skip{q}_{c}"] = np.ascontiguousarray(
                        packs[a + 1][:, c * 8:(c + 1) * 8, :].reshape(128, 8 * 128)
                    )
        in_maps.append(m)
    return in_maps


def run(in_maps, **kw):
    from concourse import bass_utils
    nc = _get_nc()
    return bass_utils.run_bass_kernel_spmd(
        nc, in_maps, core_ids=list(range(NCORES)), **kw
    )


def kernel(**inputs):
    in_maps = make_in_maps(**inputs)
    res = run(in_maps)
    return np.ascontiguousarray(
        np.concatenate([res.results[c]["out"] for c in range(NCORES)], axis=0),
        dtype=np.float32,
    )



# revision 2
# speedup vs baseline: 1.6381x; 1.6381x over previous
"""BrainRNN forward pass on 8 TRN2 NeuronCores (Bass/Tile, SPMD).

Strategy (tensor-parallel over output neurons, fp8/bf16 mixed precision):
  - Each block's 1024 output neurons are row-sharded 128/core; masks are
    folded into weights on the host.
  - Weights are stored as fp8 e3m4 scaled by 64 (entries ~N(0, 0.02^2);
    x64 keeps them in e3m4's normal range, ~1.3% RMS quant error);
    activations are bf16.  Matmuls stream the fp8 weights as the MOVING
    operand (1 cyc/row) against 64-wide bf16 stationary activations.
    Each block's sigmoid applies scale=1/64 to undo the weight scaling.
  - DMA traffic drops ~3.8x vs fp32 (13MB/core); all weight tiles are
    SBUF-resident (one tag each), loaded with large DMAs in deadline
    order split across the two HWDGE rings (sync + scalar).
  - Chain per block: sigmoid (PSUM->SBUF fp32, scale=1/64), PE transpose
    to [128(m), 64(b)], DVE copy, gpsimd cast-DMA (fp32->bf16) to DRAM,
    AllGather (bf16) over 8 cores, staggered unpack into bf16 curT.
  - Output block: each core contracts its own 128 rows of cur7 against
    W_out[:, rows].T (bf16) and writes its full [64, 512] partial; the
    host sums the 8 partials and adds b_out (replaces the ReduceScatter
    tail, which cost ~15us of exposed latency).
"""

import numpy as np

N = 8192
W = 1024
L = 8
B = 64
IN = 512
OUT = 512
NCORES = 8
RP = W // NCORES        # 128 rows per core per block
SCALE = 64.0

_BUILT = None


def _pack(A):
    """[M, K] -> [128, (K/128)*M] with packed[p, k*M+m] = A[m, k*128+p].

    Chunk kidx is A[:, kidx*128:(kidx+1)*128].T, i.e. [K=128(part), M(free)].
    """
    M, K = A.shape
    nk = K // 128
    return np.ascontiguousarray(
        A.reshape(M, nk, 128).transpose(2, 1, 0).reshape(128, nk * M)
    )


def _interleave(Pa, Pb):
    """Two packed [128, nk*128] -> [128, nk*256] with per-chunk interleave."""
    nk = Pa.shape[1] // 128
    out = np.empty((128, nk, 2, 128), Pa.dtype)
    out[:, :, 0, :] = Pa.reshape(128, nk, 128)
    out[:, :, 1, :] = Pb.reshape(128, nk, 128)
    return np.ascontiguousarray(out.reshape(128, nk * 256))


def _build():
    import concourse.bass as bass
    import concourse.bacc as bacc
    import concourse.mybir as mybir
    import concourse.tile as tile

    fp32 = mybir.dt.float32
    bf16 = mybir.dt.bfloat16
    fp8 = mybir.dt.float8e3
    AF = mybir.ActivationFunctionType

    nc = bacc.Bacc(
        "TRN2",
        target_bir_lowering=False,
        debug=False,
        enable_asserts=False,
        num_devices=NCORES,
    )

    t_hT = nc.dram_tensor("hT", [128, 64 * B], bf16, kind="ExternalInput")
    t_xT = nc.dram_tensor("xT", [128, 4 * B], bf16, kind="ExternalInput")
    t_win = nc.dram_tensor("win", [128, 4 * RP], fp8, kind="ExternalInput")
    t_rec = {
        j: nc.dram_tensor(f"rec{j}", [128, 64 * 128], fp8, kind="ExternalInput")
        for j in range(7)
    }
    t_hid = nc.dram_tensor("hid", [128, 7 * 8 * RP], fp8, kind="ExternalInput")
    # skip{q}: pair q=(2q, 2q+1) concatenated over cur-source blocks c.
    # c < 2q: both blocks interleaved, 2048 wide; c == 2q: later block only,
    # 1024 wide.  Total width 2q*2048 + 1024.
    t_skip = {
        q: nc.dram_tensor(
            f"skip{q}", [128, 2 * q * 2048 + 1024], fp8, kind="ExternalInput"
        )
        for q in range(1, 4)
    }
    t_biasrow = nc.dram_tensor("biasrow", [1, 4 * 256], bf16, kind="ExternalInput")
    t_wout = nc.dram_tensor("wout", [128, 512], bf16, kind="ExternalInput")
    t_ones = nc.dram_tensor("ones", [1, B], bf16, kind="ExternalInput")
    t_ident = nc.dram_tensor("ident", [B, B], fp32, kind="ExternalInput")
    t_out = nc.dram_tensor("out", [B, 512], fp32, kind="ExternalOutput")

    rg = [list(range(NCORES))]
    qof = lambda j: j // 2          # pair index of block j
    side = lambda j: j % 2          # column side within pair tile

    with tile.TileContext(nc) as tc:
        with (
            tc.tile_pool(name="const", bufs=1) as constp,
            tc.tile_pool(name="curs", bufs=1) as curp,
            tc.tile_pool(name="psum", bufs=1, space="PSUM") as psump,
            tc.tile_pool(name="dram", bufs=3, space="DRAM") as dramp,
        ):
            # ---- resident tiles + loads in deadline order ---------------
            # sync (SP HWDGE ring): hT + rec blocks + wout
            # scalar (ACT HWDGE ring): the rest (small + hid + skip)
            hT_sb = constp.tile([128, 64 * B], bf16, name="hT_sb", tag="hT")
            for i in range(2):
                nc.sync.dma_start(
                    out=hT_sb[:, i * 32 * B:(i + 1) * 32 * B],
                    in_=t_hT[:, i * 32 * B:(i + 1) * 32 * B],
                )
            xT_sb = constp.tile([128, 4 * B], bf16, name="xT_sb", tag="xT")
            nc.scalar.dma_start(out=xT_sb, in_=t_xT[:, :])
            win_sb = constp.tile([128, 4 * RP], fp8, name="win_sb", tag="win")
            nc.scalar.dma_start(out=win_sb, in_=t_win[:, :])
            biasrow_sb = constp.tile([1, 4 * 256], bf16, name="biasrow_sb", tag="br")
            nc.scalar.dma_start(out=biasrow_sb, in_=t_biasrow[:, :])
            ones_sb = constp.tile([1, B], bf16, name="ones_sb", tag="ones")
            nc.scalar.dma_start(out=ones_sb, in_=t_ones[:, :])
            ident_sb = constp.tile([B, B], fp32, name="ident_sb", tag="ident")
            nc.scalar.dma_start(out=ident_sb, in_=t_ident[:, :])

            rec_tiles = {}

            def load_rec(j):
                rt = constp.tile([128, 64 * 128], fp8, name=f"rec{j}t", tag=f"rec{j}")
                for i in range(2):
                    nc.sync.dma_start(
                        out=rt[:, i * 4096:(i + 1) * 4096],
                        in_=t_rec[j][:, i * 4096:(i + 1) * 4096],
                    )
                rec_tiles[j] = rt

            load_rec(0)
            load_rec(1)
            hid_sb = constp.tile([128, 7 * 8 * RP], fp8, name="hid_sb", tag="hid")
            nc.scalar.dma_start(out=hid_sb, in_=t_hid[:, :])
            load_rec(2)
            skip_sb = {}
            for q in range(1, 4):
                skip_sb[q] = constp.tile(
                    [128, 2 * q * 2048 + 1024], fp8, name=f"skip{q}t", tag=f"skip{q}"
                )
            nc.scalar.dma_start(out=skip_sb[1], in_=t_skip[1][:, :])
            load_rec(3)
            nc.scalar.dma_start(
                out=skip_sb[2][:, :4096], in_=t_skip[2][:, :4096]
            )
            nc.scalar.dma_start(
                out=skip_sb[2][:, 4096:], in_=t_skip[2][:, 4096:]
            )
            load_rec(4)
            load_rec(5)
            nc.scalar.dma_start(
                out=skip_sb[3][:, :6144], in_=t_skip[3][:, :6144]
            )
            load_rec(6)
            nc.scalar.dma_start(
                out=skip_sb[3][:, 6144:], in_=t_skip[3][:, 6144:]
            )
            wout_sb = constp.tile([128, 512], bf16, name="wout_sb", tag="wout")
            nc.sync.dma_start(out=wout_sb, in_=t_wout[:, :])

            psA = [
                psump.tile([64, 256], fp32, name=f"psA{q}", tag=f"psA{q}")
                for q in range(4)
            ]
            curT = [None] * 8

            # bias-init: psA[q] = ones.T @ biasrow[q]  (start=True opens group)
            for q in range(4):
                nc.tensor.matmul(
                    psA[q],
                    lhsT=ones_sb[:, :],
                    rhs=biasrow_sb[:, q * 256:(q + 1) * 256],
                    start=True,
                    stop=False,
                )

            def rec_mms(j):
                q, s = qof(j), side(j)
                rt = rec_tiles[j]
                for kg in range(64):
                    nc.tensor.matmul(
                        psA[q][:, s * 128:(s + 1) * 128],
                        lhsT=hT_sb[:, kg * B:(kg + 1) * B],
                        rhs=rt[:, kg * 128:(kg + 1) * 128],
                        start=False,
                        stop=False,
                    )

            def chain_tail(j):
                """sigmoid(x/64) -> transpose -> cast AllGather -> unpack."""
                q, s = qof(j), side(j)
                cp = curp.tile([64, 128], fp32, name=f"cpart{j}", tag="cpart", bufs=2)
                nc.scalar.activation(
                    cp, psA[q][:, s * 128:(s + 1) * 128], AF.Sigmoid,
                    scale=1.0 / SCALE,
                )
                pt = psump.tile([128, B], fp32, name=f"pt{j}", tag="pt", bufs=2)
                nc.tensor.transpose(pt, cp, ident_sb[:, :])
                ptsb = curp.tile([128, B], fp32, name=f"ptsb{j}", tag="ptsb", bufs=2)
                nc.vector.tensor_copy(ptsb, pt)
                agin = dramp.tile([128, B], bf16, name=f"agin{j}", tag="agin")
                agout = dramp.tile([W, B], bf16, name=f"agout{j}", tag="agout")
                nc.gpsimd.dma_start(out=agin, in_=ptsb)   # SWDGE fp32->bf16 cast
                nc.gpsimd.collective_compute(
                    "AllGather",
                    mybir.AluOpType.bypass,
                    replica_groups=rg,
                    ins=[agin.opt()],
                    outs=[agout.opt()],
                )
                dst = curp.tile([128, 8 * B], bf16, name=f"curT{j}", tag="curT",
                                bufs=6)
                for eng, kk in ((nc.gpsimd, 0), (nc.scalar, 4)):
                    eng.dma_start(
                        out=dst[:, kk * B:(kk + 4) * B].rearrange(
                            "p (k b) -> p k b", k=4
                        ),
                        in_=agout[kk * 128:(kk + 4) * 128, :].rearrange(
                            "(k p) b -> p k b", p=128
                        ),
                    )
                curT[j] = dst

            def hid_mms(j):
                """cur_{j-1} @ W_hid[j-1].T into block j's psum columns."""
                q, s = qof(j), side(j)
                base = (j - 1) * 8 * RP
                for kk in range(8):
                    nc.tensor.matmul(
                        psA[q][:, s * 128:(s + 1) * 128],
                        lhsT=curT[j - 1][:, kk * B:(kk + 1) * B],
                        rhs=hid_sb[:, base + kk * RP:base + (kk + 1) * RP],
                        start=False,
                        stop=(s == 1 and kk == 7),
                    )

            def skip_one(q, c, part=None):
                """MMs for pair q's skip from cur_c.  part: None=all columns,
                'lo'=first block's 128 cols, 'hi'=second block's 128 cols."""
                a = 2 * q
                wide = 256 if c < a else 128
                base = c * 2048 if c < a else a * 2048
                off = 0 if c < a else 128
                rlo, rwide = 0, wide
                if part == "lo" and wide == 256:
                    rlo, rwide = 0, 128
                elif part == "hi" and wide == 256:
                    rlo, rwide = 128, 128
                st = skip_sb[q]
                for kk in range(8):
                    nc.tensor.matmul(
                        psA[q][:, off + rlo:off + rlo + rwide],
                        lhsT=curT[c][:, kk * B:(kk + 1) * B],
                        rhs=st[:, base + kk * wide + rlo:
                               base + kk * wide + rlo + rwide],
                        start=False,
                        stop=False,
                    )

            # ---- block 0: x @ W_in.T + h @ Wrec(block0).T (+bias) ------
            for kk in range(4):
                nc.tensor.matmul(
                    psA[0][:, 0:128],
                    lhsT=xT_sb[:, kk * B:(kk + 1) * B],
                    rhs=win_sb[:, kk * RP:(kk + 1) * RP],
                    start=False,
                    stop=False,
                )
            rec_mms(0)
            chain_tail(0)

            rec_mms(1)                  # AG_0 window work
            hid_mms(1)
            chain_tail(1)

            rec_mms(2)                  # AG_1 window work
            skip_one(1, 0)
            skip_one(1, 1, part="lo")   # urgent: block 2's last skip source
            hid_mms(2)
            chain_tail(2)

            rec_mms(3)                  # AG_2 window work
            skip_one(1, 1, part="hi")
            skip_one(1, 2)              # urgent for block 3
            hid_mms(3)
            chain_tail(3)

            rec_mms(4)                  # AG_3 window work
            skip_one(2, 0)
            skip_one(2, 1)
            skip_one(2, 2)
            skip_one(2, 3, part="lo")   # urgent for block 4
            hid_mms(4)
            chain_tail(4)

            rec_mms(5)                  # AG_4 window work
            skip_one(3, 0)
            skip_one(2, 3, part="hi")
            skip_one(2, 4)              # urgent for block 5
            hid_mms(5)
            chain_tail(5)

            rec_mms(6)                  # AG_5 window work
            skip_one(3, 1)
            skip_one(3, 2)
            skip_one(3, 3)
            skip_one(3, 4)
            skip_one(3, 5, part="lo")   # urgent for block 6
            hid_mms(6)
            chain_tail(6)

            skip_one(3, 5, part="hi")   # AG_6 window work
            skip_one(3, 6)              # urgent for block 7
            hid_mms(7)

            # block 7 tail: sigmoid -> transpose -> local out matmul -> store
            cp7 = curp.tile([64, 128], fp32, name="cpart7", tag="cpart", bufs=2)
            nc.scalar.activation(
                cp7, psA[3][:, 128:256], AF.Sigmoid, scale=1.0 / SCALE
            )
            pt7 = psump.tile([128, B], fp32, name="pt7", tag="pt", bufs=2)
            nc.tensor.transpose(pt7, cp7, ident_sb[:, :])
            cur7T_sb = curp.tile([128, B], bf16, name="cur7T_sb", tag="c7T")
            nc.vector.tensor_copy(cur7T_sb, pt7)

            pso = psump.tile([64, 512], fp32, name="pso", tag="pso")
            nc.tensor.matmul(
                pso, lhsT=cur7T_sb, rhs=wout_sb[:, :], start=True, stop=True
            )
            out_sb = curp.tile([64, 512], fp32, name="out_sb", tag="out_sb")
            nc.vector.tensor_copy(out_sb, pso)
            nc.scalar.dma_start(out=t_out[:, :], in_=out_sb)

    nc.compile()
    return nc


def _get_nc():
    global _BUILT
    if _BUILT is None:
        _BUILT = _build()
    return _BUILT


def make_in_maps(x, hidden_states, W_in, b_in, W_hid, b_hid, W_rec, W_skip,
                 W_out, b_out, mask_hid, mask_rec, mask_skip):
    from concourse import mybir

    F8 = mybir.dt.np(mybir.dt.float8e3)
    BF = mybir.dt.np(mybir.dt.bfloat16)

    x = np.asarray(x, np.float32)
    h = np.asarray(hidden_states, np.float32)
    W_in = np.asarray(W_in, np.float32)
    b_in = np.asarray(b_in, np.float32)
    W_out = np.asarray(W_out, np.float32)
    Wh = np.asarray(W_hid, np.float32) * np.asarray(mask_hid, np.float32)
    Wr = np.asarray(W_rec, np.float32) * np.asarray(mask_rec, np.float32)
    Ws = np.asarray(W_skip, np.float32) * np.asarray(mask_skip, np.float32)
    b_hid = np.asarray(b_hid, np.float32)

    def q8(a):
        return np.clip(a * SCALE, -15.5, 15.5).astype(F8)

    hT = _pack(h).astype(BF)
    xT = _pack(x).astype(BF)
    ones = np.ones((1, B), BF)
    ident = np.eye(B, dtype=np.float32)

    in_maps = []
    for c_ in range(NCORES):
        R = slice(c_ * RP, (c_ + 1) * RP)
        biases = [b_in[R]] + [b_hid[i, R] for i in range(7)]
        biasrow = np.zeros((1, 4 * 256), np.float32)
        for j in range(8):
            biasrow[0, j * 128:(j + 1) * 128] = biases[j]
        m = {
            "hT": hT,
            "xT": xT,
            "win": _pack(q8(W_in[R])),
            "hid": np.concatenate(
                [_pack(q8(Wh[i, R])) for i in range(7)], axis=1
            ),
            "biasrow": (biasrow * SCALE).astype(BF),
            "wout": np.ascontiguousarray(W_out[:, R].T).astype(BF),
            "ones": ones,
            "ident": ident,
        }
        for j in range(7):
            m[f"rec{j}"] = _pack(q8(Wr[j, R]))
        packs = {j: _pack(q8(Ws[j - 2, R, :j * W])).reshape(128, j * 8, 128)
                 for j in range(2, 8)}
        for q in range(1, 4):
            a = 2 * q
            parts = []
            for c in range(a + 1):
                if c < a:
                    Pa = packs[a][:, c * 8:(c + 1) * 8, :].reshape(128, 8 * 128)
                    Pb = packs[a + 1][:, c * 8:(c + 1) * 8, :].reshape(128, 8 * 128)
                    parts.append(_interleave(
                        np.ascontiguousarray(Pa), np.ascontiguousarray(Pb)
                    ))
                else:
                    parts.append(np.ascontiguousarray(
                        packs[a + 1][:, c * 8:(c + 1) * 8, :].reshape(128, 8 * 128)
                    ))
            m[f"skip{q}"] = np.concatenate(parts, axis=1)
        in_maps.append(m)
    return in_maps


def run(in_maps, **kw):
    from concourse import bass_utils
    nc = _get_nc()
    return bass_utils.run_bass_kernel_spmd(
        nc, in_maps, core_ids=list(range(NCORES)), **kw
    )


def kernel(**inputs):
    b_out = np.asarray(inputs["b_out"], np.float32)
    in_maps = make_in_maps(**inputs)
    res = run(in_maps)
    acc = np.zeros((B, OUT), np.float32)
    for c in range(NCORES):
        acc += np.asarray(res.results[c]["out"], np.float32)
    return np.ascontiguousarray(acc + b_out[None, :], dtype=np.float32)


# revision 11
# speedup vs baseline: 1.7387x; 1.0614x over previous
"""BrainRNN forward pass on 8 TRN2 NeuronCores (Bass/Tile, SPMD).

Strategy (tensor-parallel over output neurons, fp8/bf16 mixed precision):
  - Each block's 1024 output neurons are row-sharded 128/core; masks are
    folded into weights on the host.
  - Weights are stored as fp8 e3m4 scaled by 64 (entries ~N(0, 0.02^2);
    x64 keeps them in e3m4's normal range, ~1.3% RMS quant error);
    activations are bf16.  Matmuls stream the fp8 weights as the MOVING
    operand (1 cyc/row) against 64-wide bf16 stationary activations.
    Each block's sigmoid applies scale=1/64 to undo the weight scaling.
  - DMA traffic drops ~3.8x vs fp32 (13MB/core); all weight tiles are
    SBUF-resident (one tag each), loaded with large DMAs in deadline
    order split across the two HWDGE rings (sync + scalar).
  - Chain per block: sigmoid (PSUM->SBUF fp32, scale=1/64), PE transpose
    to [128(m), 64(b)], DVE copy, gpsimd cast-DMA (fp32->bf16) to DRAM,
    AllGather (bf16) over 8 cores, staggered unpack into bf16 curT.
  - Output block: each core contracts its own 128 rows of cur7 against
    W_out[:, rows].T (bf16) and writes its full [64, 512] partial; the
    host sums the 8 partials and adds b_out (replaces the ReduceScatter
    tail, which cost ~15us of exposed latency).
"""

import numpy as np

N = 8192
W = 1024
L = 8
B = 64
IN = 512
OUT = 512
NCORES = 8
RP = W // NCORES        # 128 rows per core per block
SCALE = 64.0

_BUILT = None


def _pack(A):
    """[M, K] -> [128, (K/128)*M] with packed[p, k*M+m] = A[m, k*128+p].

    Chunk kidx is A[:, kidx*128:(kidx+1)*128].T, i.e. [K=128(part), M(free)].
    """
    M, K = A.shape
    nk = K // 128
    return np.ascontiguousarray(
        A.reshape(M, nk, 128).transpose(2, 1, 0).reshape(128, nk * M)
    )


def _interleave(Pa, Pb):
    """Two packed [128, nk*128] -> [128, nk*256] with per-chunk interleave."""
    nk = Pa.shape[1] // 128
    out = np.empty((128, nk, 2, 128), Pa.dtype)
    out[:, :, 0, :] = Pa.reshape(128, nk, 128)
    out[:, :, 1, :] = Pb.reshape(128, nk, 128)
    return np.ascontiguousarray(out.reshape(128, nk * 256))


def _build():
    import concourse.bass as bass
    import concourse.bacc as bacc
    import concourse.mybir as mybir
    import concourse.tile as tile

    fp32 = mybir.dt.float32
    bf16 = mybir.dt.bfloat16
    fp8 = mybir.dt.float8e3
    AF = mybir.ActivationFunctionType

    nc = bacc.Bacc(
        "TRN2",
        target_bir_lowering=False,
        debug=False,
        enable_asserts=False,
        num_devices=NCORES,
    )

    t_hT = nc.dram_tensor("hT", [128, 64 * B], bf16, kind="ExternalInput")
    t_xT = nc.dram_tensor("xT", [128, 4 * B], bf16, kind="ExternalInput")
    t_win = nc.dram_tensor("win", [128, 4 * RP], fp8, kind="ExternalInput")
    t_rec = {
        j: nc.dram_tensor(f"rec{j}", [128, 64 * 128], fp8, kind="ExternalInput")
        for j in range(7)
    }
    t_hid = nc.dram_tensor("hid", [128, 7 * 8 * RP], fp8, kind="ExternalInput")
    # skip{q}: pair q=(2q, 2q+1) concatenated over cur-source blocks c.
    # c < 2q: both blocks interleaved, 2048 wide; c == 2q: later block only,
    # 1024 wide.  Total width 2q*2048 + 1024.
    t_skip = {
        q: nc.dram_tensor(
            f"skip{q}", [128, 2 * q * 2048 + 1024], fp8, kind="ExternalInput"
        )
        for q in range(1, 4)
    }
    t_biasrow = nc.dram_tensor("biasrow", [1, 4 * 256], bf16, kind="ExternalInput")
    t_wout = nc.dram_tensor("wout", [128, 512], bf16, kind="ExternalInput")
    t_ones = nc.dram_tensor("ones", [1, B], bf16, kind="ExternalInput")
    t_ident = nc.dram_tensor("ident", [B, B], fp32, kind="ExternalInput")
    t_out = nc.dram_tensor("out", [B, 512], fp32, kind="ExternalOutput")

    rg = [list(range(NCORES))]
    qof = lambda j: j // 2          # pair index of block j
    side = lambda j: j % 2          # column side within pair tile

    with tile.TileContext(nc) as tc:
        with (
            tc.tile_pool(name="const", bufs=1) as constp,
            tc.tile_pool(name="curs", bufs=1) as curp,
            tc.tile_pool(name="psum", bufs=1, space="PSUM") as psump,
            tc.tile_pool(name="dram", bufs=3, space="DRAM") as dramp,
        ):
            # ---- resident tiles + loads in deadline order ---------------
            # sync (SP HWDGE ring): hT + rec blocks + wout
            # scalar (ACT HWDGE ring): the rest (small + hid + skip)
            hT_sb = constp.tile([128, 64 * B], bf16, name="hT_sb", tag="hT")
            for i in range(2):
                nc.sync.dma_start(
                    out=hT_sb[:, i * 32 * B:(i + 1) * 32 * B],
                    in_=t_hT[:, i * 32 * B:(i + 1) * 32 * B],
                )
            xT_sb = constp.tile([128, 4 * B], bf16, name="xT_sb", tag="xT")
            nc.scalar.dma_start(out=xT_sb, in_=t_xT[:, :])
            win_sb = constp.tile([128, 4 * RP], fp8, name="win_sb", tag="win")
            nc.scalar.dma_start(out=win_sb, in_=t_win[:, :])
            biasrow_sb = constp.tile([1, 4 * 256], bf16, name="biasrow_sb", tag="br")
            nc.scalar.dma_start(out=biasrow_sb, in_=t_biasrow[:, :])
            ones_sb = constp.tile([1, B], bf16, name="ones_sb", tag="ones")
            nc.scalar.dma_start(out=ones_sb, in_=t_ones[:, :])
            ident_sb = constp.tile([B, B], fp32, name="ident_sb", tag="ident")
            nc.scalar.dma_start(out=ident_sb, in_=t_ident[:, :])

            rec_tiles = {}

            def load_rec(j):
                rt = constp.tile([128, 64 * 128], fp8, name=f"rec{j}t", tag=f"rec{j}")
                for i in range(2):
                    nc.sync.dma_start(
                        out=rt[:, i * 4096:(i + 1) * 4096],
                        in_=t_rec[j][:, i * 4096:(i + 1) * 4096],
                    )
                rec_tiles[j] = rt

            load_rec(0)
            load_rec(1)
            hid_sb = constp.tile([128, 7 * 8 * RP], fp8, name="hid_sb", tag="hid")
            nc.scalar.dma_start(out=hid_sb, in_=t_hid[:, :])
            load_rec(2)
            skip_sb = {}
            for q in range(1, 4):
                skip_sb[q] = constp.tile(
                    [128, 2 * q * 2048 + 1024], fp8, name=f"skip{q}t", tag=f"skip{q}"
                )
            nc.scalar.dma_start(out=skip_sb[1], in_=t_skip[1][:, :])
            load_rec(3)
            nc.scalar.dma_start(
                out=skip_sb[2][:, :4096], in_=t_skip[2][:, :4096]
            )
            nc.scalar.dma_start(
                out=skip_sb[2][:, 4096:], in_=t_skip[2][:, 4096:]
            )
            load_rec(4)
            load_rec(5)
            nc.scalar.dma_start(
                out=skip_sb[3][:, :6144], in_=t_skip[3][:, :6144]
            )
            load_rec(6)
            nc.scalar.dma_start(
                out=skip_sb[3][:, 6144:], in_=t_skip[3][:, 6144:]
            )
            wout_sb = constp.tile([128, 512], bf16, name="wout_sb", tag="wout")
            nc.sync.dma_start(out=wout_sb, in_=t_wout[:, :])

            psA = [
                psump.tile([64, 256], fp32, name=f"psA{q}", tag=f"psA{q}")
                for q in range(4)
            ]
            curT = [None] * 8

            # bias-init: psA[q] = ones.T @ biasrow[q]  (start=True opens group)
            for q in range(4):
                nc.tensor.matmul(
                    psA[q],
                    lhsT=ones_sb[:, :],
                    rhs=biasrow_sb[:, q * 256:(q + 1) * 256],
                    start=True,
                    stop=False,
                )

            def rec_mms(j):
                q, s = qof(j), side(j)
                rt = rec_tiles[j]
                for kg in range(64):
                    nc.tensor.matmul(
                        psA[q][:, s * 128:(s + 1) * 128],
                        lhsT=hT_sb[:, kg * B:(kg + 1) * B],
                        rhs=rt[:, kg * 128:(kg + 1) * 128],
                        start=False,
                        stop=False,
                    )

            def chain_tail(j):
                """sigmoid(x/64) -> transpose -> cast AllGather -> unpack."""
                q, s = qof(j), side(j)
                cp = curp.tile([64, 128], fp32, name=f"cpart{j}", tag="cpart", bufs=2)
                nc.scalar.activation(
                    cp, psA[q][:, s * 128:(s + 1) * 128], AF.Sigmoid,
                    scale=1.0 / SCALE,
                )
                pt = psump.tile([128, B], fp32, name=f"pt{j}", tag="pt", bufs=2)
                nc.tensor.transpose(pt, cp, ident_sb[:, :])
                ptsb = curp.tile([128, B], fp32, name=f"ptsb{j}", tag="ptsb", bufs=2)
                nc.vector.tensor_copy(ptsb, pt)
                agin = dramp.tile([128, B], bf16, name=f"agin{j}", tag="agin")
                agout = dramp.tile([W, B], bf16, name=f"agout{j}", tag="agout")
                nc.gpsimd.dma_start(out=agin, in_=ptsb)   # SWDGE fp32->bf16 cast
                nc.gpsimd.collective_compute(
                    "AllGather",
                    mybir.AluOpType.bypass,
                    replica_groups=rg,
                    ins=[agin.opt()],
                    outs=[agout.opt()],
                )
                dst = curp.tile([128, 8 * B], bf16, name=f"curT{j}", tag="curT",
                                bufs=6)
                for eng, kk in ((nc.gpsimd, 0), (nc.scalar, 4)):
                    eng.dma_start(
                        out=dst[:, kk * B:(kk + 4) * B].rearrange(
                            "p (k b) -> p k b", k=4
                        ),
                        in_=agout[kk * 128:(kk + 4) * 128, :].rearrange(
                            "(k p) b -> p k b", p=128
                        ),
                    )
                curT[j] = dst

            def hid_mms(j):
                """cur_{j-1} @ W_hid[j-1].T into block j's psum columns."""
                q, s = qof(j), side(j)
                base = (j - 1) * 8 * RP
                for kk in range(8):
                    nc.tensor.matmul(
                        psA[q][:, s * 128:(s + 1) * 128],
                        lhsT=curT[j - 1][:, kk * B:(kk + 1) * B],
                        rhs=hid_sb[:, base + kk * RP:base + (kk + 1) * RP],
                        start=False,
                        stop=(s == 1 and kk == 7),
                    )

            def skip_one(q, c, part=None):
                """MMs for pair q's skip from cur_c.  part: None=all columns,
                'lo'=first block's 128 cols, 'hi'=second block's 128 cols."""
                a = 2 * q
                wide = 256 if c < a else 128
                base = c * 2048 if c < a else a * 2048
                off = 0 if c < a else 128
                rlo, rwide = 0, wide
                if part == "lo" and wide == 256:
                    rlo, rwide = 0, 128
                elif part == "hi" and wide == 256:
                    rlo, rwide = 128, 128
                st = skip_sb[q]
                for kk in range(8):
                    nc.tensor.matmul(
                        psA[q][:, off + rlo:off + rlo + rwide],
                        lhsT=curT[c][:, kk * B:(kk + 1) * B],
                        rhs=st[:, base + kk * wide + rlo:
                               base + kk * wide + rlo + rwide],
                        start=False,
                        stop=False,
                    )

            # ---- block 0: x @ W_in.T + h @ Wrec(block0).T (+bias) ------
            for kk in range(4):
                nc.tensor.matmul(
                    psA[0][:, 0:128],
                    lhsT=xT_sb[:, kk * B:(kk + 1) * B],
                    rhs=win_sb[:, kk * RP:(kk + 1) * RP],
                    start=False,
                    stop=False,
                )
            rec_mms(0)
            chain_tail(0)

            rec_mms(1)                  # AG_0 window work
            hid_mms(1)
            chain_tail(1)

            rec_mms(2)                  # AG_1 window work
            skip_one(1, 0)
            skip_one(1, 1, part="lo")   # urgent: block 2's last skip source
            hid_mms(2)
            chain_tail(2)

            rec_mms(3)                  # AG_2 window work
            skip_one(1, 1, part="hi")
            skip_one(1, 2)              # urgent for block 3
            hid_mms(3)
            chain_tail(3)

            rec_mms(4)                  # AG_3 window work
            skip_one(2, 0)
            skip_one(2, 1)
            skip_one(2, 2)
            skip_one(2, 3, part="lo")   # urgent for block 4
            hid_mms(4)
            chain_tail(4)

            rec_mms(5)                  # AG_4 window work
            skip_one(3, 0)
            skip_one(2, 3, part="hi")
            skip_one(2, 4)              # urgent for block 5
            hid_mms(5)
            chain_tail(5)

            rec_mms(6)                  # AG_5 window work
            skip_one(3, 1)
            skip_one(3, 2)
            skip_one(3, 3)
            skip_one(3, 4)
            skip_one(3, 5, part="lo")   # urgent for block 6
            hid_mms(6)
            chain_tail(6)

            skip_one(3, 5, part="hi")   # AG_6 window work
            skip_one(3, 6)              # urgent for block 7
            hid_mms(7)

            # block 7 tail: sigmoid -> transpose -> local out matmul -> store
            cp7 = curp.tile([64, 128], fp32, name="cpart7", tag="cpart", bufs=2)
            nc.scalar.activation(
                cp7, psA[3][:, 128:256], AF.Sigmoid, scale=1.0 / SCALE
            )
            pt7 = psump.tile([128, B], fp32, name="pt7", tag="pt", bufs=2)
            nc.tensor.transpose(pt7, cp7, ident_sb[:, :])
            cur7T_sb = curp.tile([128, B], bf16, name="cur7T_sb", tag="c7T")
            nc.vector.tensor_copy(cur7T_sb, pt7)

            pso = psump.tile([64, 512], fp32, name="pso", tag="pso")
            nc.tensor.matmul(
                pso, lhsT=cur7T_sb, rhs=wout_sb[:, :], start=True, stop=True
            )
            out_sb = curp.tile([64, 512], fp32, name="out_sb", tag="out_sb")
            nc.vector.tensor_copy(out_sb, pso)
            nc.scalar.dma_start(out=t_out[:, :], in_=out_sb)

    nc.compile()
    return nc


def _get_nc():
    global _BUILT
    if _BUILT is None:
        _BUILT = _build()
    return _BUILT


def make_in_maps(x, hidden_states, W_in, b_in, W_hid, b_hid, W_rec, W_skip,
                 W_out, b_out, mask_hid, mask_rec, mask_skip):
    from concourse import mybir

    F8 = mybir.dt.np(mybir.dt.float8e3)
    BF = mybir.dt.np(mybir.dt.bfloat16)

    x = np.asarray(x, np.float32)
    h = np.asarray(hidden_states, np.float32)
    W_in = np.asarray(W_in, np.float32)
    b_in = np.asarray(b_in, np.float32)
    W_out = np.asarray(W_out, np.float32)
    Wh = np.asarray(W_hid, np.float32) * np.asarray(mask_hid, np.float32)
    Wr = np.asarray(W_rec, np.float32) * np.asarray(mask_rec, np.float32)
    Ws = np.asarray(W_skip, np.float32) * np.asarray(mask_skip, np.float32)
    b_hid = np.asarray(b_hid, np.float32)

    def q8(a):
        return np.clip(a * SCALE, -15.5, 15.5).astype(F8)

    hT = _pack(h).astype(BF)
    xT = _pack(x).astype(BF)
    ones = np.ones((1, B), BF)
    ident = np.eye(B, dtype=np.float32)

    in_maps = []
    for c_ in range(NCORES):
        R = slice(c_ * RP, (c_ + 1) * RP)
        biases = [b_in[R]] + [b_hid[i, R] for i in range(7)]
        biasrow = np.zeros((1, 4 * 256), np.float32)
        for j in range(8):
            biasrow[0, j * 128:(j + 1) * 128] = biases[j]
        m = {
            "hT": hT,
            "xT": xT,
            "win": _pack(q8(W_in[R])),
            "hid": np.concatenate(
                [_pack(q8(Wh[i, R])) for i in range(7)], axis=1
            ),
            "biasrow": (biasrow * SCALE).astype(BF),
            "wout": np.ascontiguousarray(W_out[:, R].T).astype(BF),
            "ones": ones,
            "ident": ident,
        }
        for j in range(7):
            m[f"rec{j}"] = _pack(q8(Wr[j, R]))
        packs = {j: _pack(q8(Ws[j - 2, R, :j * W])).reshape(128, j * 8, 128)
                 for j in range(2, 8)}
        for q in range(1, 4):
            a = 2 * q
            parts = []
            for c in range(a + 1):
                if c < a:
                    Pa = packs[a][:, c * 8:(c + 1) * 8, :].reshape(128, 8 * 128)
                    Pb = packs[a + 1][:, c * 8:(c + 1) * 8, :].reshape(128, 8 * 128)
                    parts.append(_interleave(
                        np.ascontiguousarray(Pa), np.ascontiguousarray(Pb)
                    ))
                else:
                    parts.append(np.ascontiguousarray(
                        packs[a + 1][:, c * 8:(c + 1) * 8, :].reshape(128, 8 * 128)
                    ))
            m[f"skip{q}"] = np.concatenate(parts, axis=1)
        in_maps.append(m)
    return in_maps


def run(in_maps, **kw):
    from concourse import bass_utils
    nc = _get_nc()
    return bass_utils.run_bass_kernel_spmd(
        nc, in_maps, core_ids=list(range(NCORES)), **kw
    )


def kernel(**inputs):
    b_out = np.asarray(inputs["b_out"], np.float32)
    in_maps = make_in_maps(**inputs)
    res = run(in_maps)
    acc = np.zeros((B, OUT), np.float32)
    for c in range(NCORES):
        acc += np.asarray(res.results[c]["out"], np.float32)
    return np.ascontiguousarray(acc + b_out[None, :], dtype=np.float32)
